# revision 26
# baseline (speedup 1.0000x reference)
"""Causal multi-head attention (B=4, T=2048, C=1024, H=16, HD=64) on 8 trn2 cores.

Sharding: core i -> batch b = i//2, head-half hh = i%2 (8 heads = 512 dims).
Each core computes q/k/v projections for its 512 head-dims, causal attention
for its 8 heads, and its partial of the output projection (Wo column block).
Host sums the two head-half partials per batch.

Dataflow (all matmul operands bf16, PSUM accumulation f32):
  - x^T resident in SBUF as 8 [128c, 2048t] chunks (bf16).
  - q/k projected per (pair, t-chunk) into [128d, 512t] tiles; v projected
    directly in [t, d] layout ([128t, 512d] psum tiles) and scattered into
    vAB[chunk] = [128tk, jt(4), head(8), 128] where cols 0:64 hold v and
    cols 64:128 hold ones (so the PV matmul also emits the softmax
    denominator broadcast across psum partitions 64:128).
  - Attention runs query-chunk (jq) outer, S-transposed: S[tk, tq] psum
    [128, 1024] tiles (2 tk-tiles x 1 head), exp on ACT (scale=1/8 fused,
    bf16 out), causal mask on the 4 diagonal tiles via DVE multiply,
    PV accumulation into pa[128, 512] (64 att dims + 64 denom rows).
    Normalize = reciprocal_approx_fast(denom) + DVE multiply -> attT tile
    (SBUF bf16, no DRAM spill).
  - Wo phase per jq (interleaved with the next chunk's projections by the
    tile scheduler): y[tt, ch] accumulated over the 4 pairs from SBUF attT.
"""

import sys
from contextlib import ExitStack

import numpy as np
import ml_dtypes

try:
    from concourse import bass, tile, mybir
except ImportError:  # pragma: no cover
    sys.path.insert(0, "/opt/trn_rl_repo")
    from concourse import bass, tile, mybir

from concourse.bass2jax import _bass_exec_p, install_neuronx_cc_hook

F32 = mybir.dt.float32
BF16 = mybir.dt.bfloat16
AF = mybir.ActivationFunctionType
ALU = mybir.AluOpType

B, T, C = 4, 2048, 1024
H, HD = 16, 64
NCORES = 8
HH = 512          # head-dims per core (8 heads)
NPAIR = 4         # head-pairs per core (128 dims each)
NCC = C // 128    # 8 contraction chunks for projections
NCH = T // 512    # 4 t/tk chunks of 512
NTT = T // 128    # 16 t-tiles

_PROGRAM = None


def _build_program():
    nc = bass.Bass("TRN2", target_bir_lowering=False, debug=False)

    xT_d = nc.declare_dram_parameter("xT", [C, T], BF16, isOutput=False)
    wq_d = nc.declare_dram_parameter("wq", [128, NCC * HH], BF16, isOutput=False)
    wk_d = nc.declare_dram_parameter("wk", [128, NCC * HH], BF16, isOutput=False)
    wv_d = nc.declare_dram_parameter("wv", [128, NCC * HH], BF16, isOutput=False)
    wo_d = nc.declare_dram_parameter("wo", [128, NPAIR * 2 * 512], BF16, isOutput=False)
    mk_d = nc.declare_dram_parameter("mk", [128, 128], BF16, isOutput=False)
    y_d = nc.declare_dram_parameter("y", [T, C], F32, isOutput=True)

    xT = xT_d.ap()
    y = y_d.ap()

    with tile.TileContext(nc) as tc, ExitStack() as ctx:
        cst = ctx.enter_context(tc.tile_pool(name="cst", bufs=1))
        qp = ctx.enter_context(tc.tile_pool(name="qp", bufs=8))
        ap_ = ctx.enter_context(tc.tile_pool(name="ap", bufs=12))
        ep = ctx.enter_context(tc.tile_pool(name="ep", bufs=8))
        rp = ctx.enter_context(tc.tile_pool(name="rp", bufs=2))
        yp = ctx.enter_context(tc.tile_pool(name="yp", bufs=2))
        psS = ctx.enter_context(tc.tile_pool(name="psS", bufs=2, space="PSUM"))
        psPA = ctx.enter_context(tc.tile_pool(name="psPA", bufs=1, space="PSUM"))
        psA = ctx.enter_context(tc.tile_pool(name="psA", bufs=2, space="PSUM"))

        # --- static SBUF: weights, masks, x chunks, k/v tiles
        wq_s = cst.tile([128, NCC * HH], BF16, tag="wq_s")
        wk_s = cst.tile([128, NCC * HH], BF16, tag="wk_s")
        wv_s = cst.tile([128, NCC * HH], BF16, tag="wv_s")
        wo_s = cst.tile([128, NPAIR * 2 * 512], BF16, tag="wo_s")
        tri_s = cst.tile([128, 128], BF16, tag="tri_s")
        for sb, dr in ((wq_s, wq_d), (wk_s, wk_d), (wv_s, wv_d), (wo_s, wo_d), (tri_s, mk_d)):
            nc.sync.dma_start(sb[:], dr.ap()[:])

        # x chunks split per (row-chunk cc, t-chunk c) so projections can
        # start as soon as the columns they touch have landed. DMA order:
        # t-chunk NCH-1 first (q proj of the heaviest query chunk, which runs
        # first), then 0, 1, 2 (k/v in tk order).
        xts = [[None] * NCH for _ in range(NCC)]
        for c in range(NCH):
            for cc in range(NCC):
                xt = cst.tile([128, 512], BF16, tag=f"x{cc}_{c}", name=f"x{cc}_{c}")
                # issued from GpSimd so descriptor generation runs in
                # parallel with the Sync engine's weight DMAs
                nc.gpsimd.dma_start(
                    xt[:], xT[cc * 128:(cc + 1) * 128, c * 512:(c + 1) * 512]
                )
                xts[cc][c] = xt

        # kT[p][c]: [128 d-pair, 512 tk]; vAB[c]: [128 tk, jt(4)*head(8)*128]
        kT = [[None] * NCH for _ in range(NPAIR)]
        for p in range(NPAIR):
            for c in range(NCH):
                kT[p][c] = cst.tile([128, 512], BF16, tag=f"k{p}{c}", name=f"k{p}{c}")
        vAB = []
        for c in range(NCH):
            v_t = cst.tile([128, 4 * 8 * 128], BF16, tag=f"v{c}", name=f"v{c}")
            vAB.append(v_t)
            # ones columns 64:128 of each [*, jt, h, 128] block
            nc.vector.memset(v_t[:].rearrange("q (a b) -> q a b", b=128)[:, :, 64:128], 1.0)

        def proj_qk(w_s, p, c, dest):
            acc = psA.tile([128, 512], F32, tag="acc", name="acc")
            for cc in range(NCC):
                nc.tensor.matmul(
                    acc[:],
                    w_s[:, cc * HH + p * 128: cc * HH + (p + 1) * 128],
                    xts[cc][c][:],
                    start=(cc == 0),
                    stop=(cc == NCC - 1),
                )
            nc.vector.tensor_copy(dest, acc[:])

        def proj_v(c):
            for jt in range(4):
                acc = psA.tile([128, 512], F32, tag="acc", name="acc")
                for cc in range(NCC):
                    nc.tensor.matmul(
                        acc[:],
                        xts[cc][c][:, jt * 128:(jt + 1) * 128],
                        wv_s[:, cc * HH:(cc + 1) * HH],
                        start=(cc == 0),
                        stop=(cc == NCC - 1),
                    )
                dst = vAB[c][:].rearrange("q (a b c2) -> q a b c2", a=4, b=8)[:, jt, :, 0:64]
                nc.vector.tensor_copy(dst, acc[:])

        def proj_chunk_kv(c):
            for p in range(NPAIR):
                proj_qk(wk_s, p, c, kT[p][c][:])
            proj_v(c)

        def proj_chunk_q(jq):
            qs = []
            for p in range(NPAIR):
                qt = qp.tile([128, 512], BF16, tag="q", name="qt")
                proj_qk(wq_s, p, jq, qt[:])
                qs.append(qt)
            return qs

        def wo_emit(jq, att_list):
            for ch in range(2):
                for tt in range(4):
                    y_ps = psA.tile([128, 512], F32, tag="acc", name="y_ps")
                    for p in range(NPAIR):
                        nc.tensor.matmul(
                            y_ps[:],
                            att_list[p][:, tt * 128:(tt + 1) * 128],
                            wo_s[:, (p * 2 + ch) * 512:(p * 2 + ch + 1) * 512],
                            start=(p == 0),
                            stop=(p == NPAIR - 1),
                        )
                    yb = yp.tile([128, 512], F32, tag="yb", name="yb")
                    nc.vector.tensor_copy(yb[:], y_ps[:])
                    t0 = jq * 512 + tt * 128
                    nc.sync.dma_start(
                        y[t0:t0 + 128, ch * 512:(ch + 1) * 512], yb[:]
                    )

        # Emission order doubles as scheduling priority, so PE filler work
        # (projections, Wo) is spread to where the scalar engine would
        # otherwise outpace the PE: each section emits the next chunk's
        # projections, and the last (heaviest) section absorbs kv(3), Wo(2)
        # and Wo(3).
        # Section 0 projections are emitted per-pair just-in-time so the
        # scalar exp pipeline starts as early as possible.
        qs = [None] * NPAIR
        att_saved = {}
        pending = []

        def norm_flush(item):
            att_t, stash = item
            for h, (ar, dn) in enumerate(stash):
                lt = rp.tile([64, 512], F32, tag="lt", name="lt")
                rc = rp.tile([64, 512], F32, tag="rc", name="rc")
                nc.scalar.activation(lt[:], dn[:], AF.Ln)
                nc.scalar.activation(rc[:], lt[:], AF.Exp, scale=-1.0)
                nc.vector.tensor_tensor(
                    att_t[h * 64:(h + 1) * 64, :], ar[:], rc[:], ALU.mult
                )

        for jq in range(NCH):
            att_cur = []
            for p in range(NPAIR):
                if jq == 0:
                    proj_qk(wk_s, p, 0, kT[p][0][:])
                    qt = qp.tile([128, 512], BF16, tag="q", name="qt")
                    proj_qk(wq_s, p, 0, qt[:])
                    qs[p] = qt
                    if p == 0:
                        proj_v(0)
                if jq == NCH - 1 and p == 2:
                    wo_emit(NCH - 2, att_saved.pop(NCH - 2))
                if len(pending) > 1:
                    norm_flush(pending.pop(0))
                paA = psPA.tile([128, 512], F32, tag="paA", name="paA")
                paB = psPA.tile([128, 512], F32, tag="paB", name="paB")
                att_t = ap_.tile([128, 512], BF16, tag="att", name="att_t")
                # chunks: (tile, col0, ncol, e_off) - diagonal tiles trimmed
                # to their causal column range [128d, 512) plus a [128,128]
                # triangle mask on the first 128 columns.
                chunks = [
                    ([(2 * m, 0, 512, 0), (2 * m + 1, 0, 512, 512)], 1024, False)
                    for m in range(2 * jq)
                ]
                t0_ = 4 * jq
                chunks.append(
                    ([(t0_, 0, 512, 0), (t0_ + 1, 128, 384, 512)], 896, True)
                )
                chunks.append(
                    ([(t0_ + 2, 256, 256, 0), (t0_ + 3, 384, 128, 256)], 384, True)
                )
                nchk = len(chunks)
                for ci, (parts, width, diag) in enumerate(chunks):
                    if jq == NCH - 1 and p == 0 and ci == nchk - 2:
                        proj_chunk_kv(NCH - 1)  # filler for the heavy section
                    for h in range(2):
                        pa = paA if h == 0 else paB
                        s = psS.tile([128, width], F32, tag="s", name="s")
                        for tl, col0, ncol, eo in parts:
                            nc.tensor.matmul(
                                s[:, eo:eo + ncol],
                                kT[p][tl // 4][h * 64:(h + 1) * 64,
                                               (tl % 4) * 128:(tl % 4 + 1) * 128],
                                qs[p][h * 64:(h + 1) * 64, col0:col0 + ncol],
                                start=True,
                                stop=True,
                            )
                        e = ep.tile([128, width], BF16, tag="e", name="e")
                        nc.scalar.activation(e[:], s[:], AF.Exp, scale=0.125)
                        if diag:
                            # zero each tile's intra-tile upper triangle
                            # (its first 128 columns)
                            for _, _, _, eo in parts:
                                nc.vector.tensor_tensor(
                                    e[:, eo:eo + 128], e[:, eo:eo + 128],
                                    tri_s[:], ALU.mult,
                                )
                        for pi, (tl, col0, ncol, eo) in enumerate(parts):
                            vsl = vAB[tl // 4][:].rearrange(
                                "q (a b c2) -> q a b c2", a=4, b=8
                            )[:, tl % 4, p * 2 + h, :]
                            nc.tensor.matmul(
                                pa[:, col0:col0 + ncol],
                                vsl,
                                e[:, eo:eo + ncol],
                                start=(ci == 0 and pi == 0),
                                stop=(ci == nchk - 1 and tl == t0_ + 3),
                            )
                # Stash raw att + denominator to SBUF with two fast DVE
                # copies so the pa psum slots free in ~1.4us (the next pair's
                # PV is serialized on them). The actual normalize
                # (r = exp(-ln(d)) on ACT, then DVE multiply) is emitted one
                # pair later so it slots into the ACT FIFO without stalling
                # the attention pipeline.
                stash = []
                for h, pa in ((0, paA), (1, paB)):
                    ar = rp.tile([64, 512], BF16, tag="ar", name="ar", bufs=4)
                    dn = rp.tile([64, 512], F32, tag="dn", name="dn", bufs=4)
                    nc.vector.tensor_copy(ar[:], pa[0:64, :])
                    nc.vector.tensor_copy(dn[:], pa[64:128, :])
                    stash.append((ar, dn))
                pending.append((att_t, stash))
                att_cur.append(att_t)

            while pending:
                norm_flush(pending.pop(0))

            if jq + 1 < NCH:
                qs = proj_chunk_q(jq + 1)
                if jq + 1 < NCH - 1:
                    proj_chunk_kv(jq + 1)  # kv(3) deferred into section 3

            if jq == NCH - 2:
                att_saved[jq] = att_cur  # Wo(2) emitted inside section 3
            else:
                wo_emit(jq, att_cur)

    _split_matmul_waits(nc)
    return nc


def _split_matmul_waits(nc):
    """walrus's fused-LDW matmul lowering can't carry multiple sync waits
    (S3_LW setupSyncWait assert). Move every matmul's waits onto a
    preceding same-engine NoOp, which lowers with full sync support."""
    f = nc.m.functions[0]
    k = 0
    for bb in f.blocks:
        insts = bb.instructions
        out = []
        for i in insts:
            waits = list(i.sync_info.on_wait) if i.sync_info is not None else []
            keep = 0 if type(i).__name__ == "InstMatmult" else 1
            if len(waits) > keep:
                moved, kept = waits[: len(waits) - keep], waits[len(waits) - keep:]
                for w in moved:
                    n = mybir.InstNoOp(name=f"I-mmwait{k}")
                    k += 1
                    n.engine = i.engine
                    n.sync_info = mybir.SyncInfo(on_wait=[w], on_update=[])
                    nc.register_instruction(n)
                    out.append(n)
                i.sync_info = mybir.SyncInfo(
                    on_wait=kept, on_update=list(i.sync_info.on_update)
                )
            out.append(i)
        if k:
            bb.instructions = out
    return nc


def _get_program():
    global _PROGRAM
    if _PROGRAM is None:
        _PROGRAM = _build_program()
    return _PROGRAM


_RUNNER = None


def _get_runner():
    """Compile the SPMD program into a cached sharded jit callable."""
    global _RUNNER
    if _RUNNER is not None:
        return _RUNNER
    import jax
    from jax.experimental.shard_map import shard_map
    from jax.sharding import Mesh, PartitionSpec

    nc = _get_program()
    install_neuronx_cc_hook()

    partition_name = (
        nc.partition_id_tensor.name if nc.partition_id_tensor else None
    )
    in_names, out_names, out_avals = [], [], []
    for alloc in nc.m.functions[0].allocations:
        if not isinstance(alloc, mybir.MemoryLocationSet):
            continue
        name = alloc.memorylocations[0].name
        if alloc.kind == "ExternalInput":
            if name != partition_name:
                in_names.append(name)
        elif alloc.kind == "ExternalOutput":
            out_names.append(name)
            out_avals.append(
                jax.core.ShapedArray(tuple(alloc.tensor_shape), mybir.dt.np(alloc.dtype))
            )
    n_params = len(in_names)
    zero_outs = [np.zeros(a.shape, a.dtype) for a in out_avals]
    all_in_names = list(in_names) + list(out_names)
    if partition_name is not None:
        all_in_names.append(partition_name)
    all_in_names = tuple(all_in_names)

    def _body(*args):
        operands = list(args)
        if partition_name is not None:
            from concourse.bass2jax import partition_id_tensor

            operands.append(partition_id_tensor())
        outs = _bass_exec_p.bind(
            *operands,
            out_avals=tuple(out_avals),
            in_names=all_in_names,
            out_names=tuple(out_names),
            lowering_input_output_aliases=(),
            sim_require_finite=True,
            sim_require_nnan=True,
            nc=nc,
        )
        return tuple(outs)

    devices = jax.devices()[:NCORES]
    assert len(devices) == NCORES, devices
    mesh = Mesh(np.asarray(devices), ("core",))
    n_all = n_params + len(out_names)
    sharded = jax.jit(
        shard_map(
            _body,
            mesh=mesh,
            in_specs=(PartitionSpec("core"),) * n_all,
            out_specs=(PartitionSpec("core"),) * len(out_names),
            check_rep=False,
        ),
        keep_unused=True,
    )
    _RUNNER = dict(
        sharded=sharded,
        in_names=in_names,
        out_names=out_names,
        out_avals=out_avals,
        zero_outs=zero_outs,
        mesh=mesh,
    )
    return _RUNNER


def _run(in_maps):
    r = _get_runner()
    concat_in = [
        np.concatenate([np.asarray(m[name]) for m in in_maps], axis=0)
        for name in r["in_names"]
    ]
    concat_zeros = [
        np.zeros((NCORES * z.shape[0], *z.shape[1:]), z.dtype) for z in r["zero_outs"]
    ]
    out_arrs = r["sharded"](*concat_in, *concat_zeros)
    return [
        {
            name: np.asarray(out_arrs[i]).reshape(NCORES, *r["out_avals"][i].shape)[c]
            for i, name in enumerate(r["out_names"])
        }
        for c in range(NCORES)
    ]


def timed_run(in_maps, iters=10):
    """Execute with inputs pre-staged on device; return per-iteration seconds."""
    import time
    import jax

    r = _get_runner()
    concat_in = [
        np.concatenate([np.asarray(m[name]) for m in in_maps], axis=0)
        for name in r["in_names"]
    ]
    concat_zeros = [
        np.zeros((NCORES * z.shape[0], *z.shape[1:]), z.dtype) for z in r["zero_outs"]
    ]
    from jax.sharding import NamedSharding, PartitionSpec

    sh = NamedSharding(r["mesh"], PartitionSpec("core"))
    args = [jax.device_put(a, sh) for a in concat_in + concat_zeros]
    out = r["sharded"](*args)  # warmup + compile
    jax.block_until_ready(out)
    times = []
    for _ in range(iters):
        t0 = time.perf_counter()
        out = r["sharded"](*args)
        jax.block_until_ready(out)
        times.append(time.perf_counter() - t0)
    return times


def bf16(a):
    return np.asarray(a, dtype=np.float32).astype(ml_dtypes.bfloat16)


def make_masks():
    pidx = np.arange(128)[:, None]
    fidx = np.arange(128)[None, :]
    return bf16((pidx <= fidx).astype(np.float32))


def make_in_maps(x, Wq, Wk, Wv, Wo):
    x = np.asarray(x, dtype=np.float32)
    Wq = np.asarray(Wq, dtype=np.float32)
    Wk = np.asarray(Wk, dtype=np.float32)
    Wv = np.asarray(Wv, dtype=np.float32)
    Wo = np.asarray(Wo, dtype=np.float32)
    mk = make_masks()
    xTs = [bf16(np.ascontiguousarray(x[b].T)) for b in range(B)]

    def wslices(W, sl):
        # [C, 512] (c, d) -> [128, cc(8), 512] -> [128, 4096]
        wt = np.ascontiguousarray(W[sl, :].T)  # torch linear: y = x @ W.T
        return bf16(wt.reshape(NCC, 128, HH).transpose(1, 0, 2).reshape(128, NCC * HH))

    in_maps = []
    for core in range(NCORES):
        b, hh = core // 2, core % 2
        sl = slice(hh * HH, (hh + 1) * HH)
        woT = np.ascontiguousarray(Wo[:, sl].T)  # [512 hd, 1024 c]
        wo_t = bf16(
            woT.reshape(NPAIR, 128, 2, 512).transpose(1, 0, 2, 3).reshape(128, NPAIR * 1024)
        )
        in_maps.append({
            "xT": xTs[b],
            "wq": wslices(Wq, sl),
            "wk": wslices(Wk, sl),
            "wv": wslices(Wv, sl),
            "wo": wo_t,
            "mk": mk,
        })
    return in_maps


def kernel(x, Wq, Wk, Wv, Wo):
    in_maps = make_in_maps(x, Wq, Wk, Wv, Wo)
    results = _run(in_maps)
    out = np.empty((B, T, C), dtype=np.float32)
    for b in range(B):
        out[b] = results[2 * b]["y"] + results[2 * b + 1]["y"]
    return out


# revision 27
# speedup vs baseline: 1.0131x; 1.0131x over previous
"""Causal multi-head attention (B=4, T=2048, C=1024, H=16, HD=64) on 8 trn2 cores.

Sharding: core i -> batch b = i//2, head-half hh = i%2 (8 heads = 512 dims).
Each core computes q/k/v projections for its 512 head-dims, causal attention
for its 8 heads, and its partial of the output projection (Wo column block).
Host sums the two head-half partials per batch.

Dataflow (all matmul operands bf16, PSUM accumulation f32):
  - x^T resident in SBUF as 8 [128c, 2048t] chunks (bf16).
  - q/k projected per (pair, t-chunk) into [128d, 512t] tiles; v projected
    directly in [t, d] layout ([128t, 512d] psum tiles) and scattered into
    vAB[chunk] = [128tk, jt(4), head(8), 128] where cols 0:64 hold v and
    cols 64:128 hold ones (so the PV matmul also emits the softmax
    denominator broadcast across psum partitions 64:128).
  - Attention runs query-chunk (jq) outer, S-transposed: S[tk, tq] psum
    [128, 1024] tiles (2 tk-tiles x 1 head), exp on ACT (scale=1/8 fused,
    bf16 out), causal mask on the 4 diagonal tiles via DVE multiply,
    PV accumulation into pa[128, 512] (64 att dims + 64 denom rows).
    Normalize = reciprocal_approx_fast(denom) + DVE multiply -> attT tile
    (SBUF bf16, no DRAM spill).
  - Wo phase per jq (interleaved with the next chunk's projections by the
    tile scheduler): y[tt, ch] accumulated over the 4 pairs from SBUF attT.
"""

import sys
from contextlib import ExitStack

import numpy as np
import ml_dtypes

try:
    from concourse import bass, tile, mybir
except ImportError:  # pragma: no cover
    sys.path.insert(0, "/opt/trn_rl_repo")
    from concourse import bass, tile, mybir

from concourse.bass2jax import _bass_exec_p, install_neuronx_cc_hook

F32 = mybir.dt.float32
BF16 = mybir.dt.bfloat16
AF = mybir.ActivationFunctionType
ALU = mybir.AluOpType

B, T, C = 4, 2048, 1024
H, HD = 16, 64
NCORES = 8
HH = 512          # head-dims per core (8 heads)
NPAIR = 4         # head-pairs per core (128 dims each)
NCC = C // 128    # 8 contraction chunks for projections
NCH = T // 512    # 4 t/tk chunks of 512
NTT = T // 128    # 16 t-tiles

_PROGRAM = None


def _build_program():
    nc = bass.Bass("TRN2", target_bir_lowering=False, debug=False)

    xT_d = nc.declare_dram_parameter("xT", [C, T], BF16, isOutput=False)
    wq_d = nc.declare_dram_parameter("wq", [128, NCC * HH], BF16, isOutput=False)
    wk_d = nc.declare_dram_parameter("wk", [128, NCC * HH], BF16, isOutput=False)
    wv_d = nc.declare_dram_parameter("wv", [128, NCC * HH], BF16, isOutput=False)
    wo_d = nc.declare_dram_parameter("wo", [128, NPAIR * 2 * 512], BF16, isOutput=False)
    mk_d = nc.declare_dram_parameter("mk", [128, 128], BF16, isOutput=False)
    y_d = nc.declare_dram_parameter("y", [T, C], F32, isOutput=True)

    xT = xT_d.ap()
    y = y_d.ap()

    with tile.TileContext(nc) as tc, ExitStack() as ctx:
        cst = ctx.enter_context(tc.tile_pool(name="cst", bufs=1))
        qp = ctx.enter_context(tc.tile_pool(name="qp", bufs=8))
        ap_ = ctx.enter_context(tc.tile_pool(name="ap", bufs=12))
        ep = ctx.enter_context(tc.tile_pool(name="ep", bufs=6))
        rp = ctx.enter_context(tc.tile_pool(name="rp", bufs=2))
        yp = ctx.enter_context(tc.tile_pool(name="yp", bufs=2))
        psS = ctx.enter_context(tc.tile_pool(name="psS", bufs=2, space="PSUM"))
        psPA = ctx.enter_context(tc.tile_pool(name="psPA", bufs=1, space="PSUM"))
        psA = ctx.enter_context(tc.tile_pool(name="psA", bufs=2, space="PSUM"))

        # --- static SBUF: weights, masks, x chunks, k/v tiles
        wq_s = cst.tile([128, NCC * HH], BF16, tag="wq_s")
        wk_s = cst.tile([128, NCC * HH], BF16, tag="wk_s")
        wv_s = cst.tile([128, NCC * HH], BF16, tag="wv_s")
        wo_s = cst.tile([128, NPAIR * 2 * 512], BF16, tag="wo_s")
        tri_s = cst.tile([128, 128], BF16, tag="tri_s")
        for sb, dr in ((wq_s, wq_d), (wk_s, wk_d), (wv_s, wv_d), (wo_s, wo_d), (tri_s, mk_d)):
            nc.sync.dma_start(sb[:], dr.ap()[:])

        # x chunks split per (row-chunk cc, t-chunk c) so projections can
        # start as soon as the columns they touch have landed. DMA order:
        # t-chunk NCH-1 first (q proj of the heaviest query chunk, which runs
        # first), then 0, 1, 2 (k/v in tk order).
        xts = [[None] * NCH for _ in range(NCC)]
        for c in range(NCH):
            for cc in range(NCC):
                xt = cst.tile([128, 512], BF16, tag=f"x{cc}_{c}", name=f"x{cc}_{c}")
                # issued from GpSimd so descriptor generation runs in
                # parallel with the Sync engine's weight DMAs
                nc.gpsimd.dma_start(
                    xt[:], xT[cc * 128:(cc + 1) * 128, c * 512:(c + 1) * 512]
                )
                xts[cc][c] = xt

        # kT[p][c]: [128 d-pair, 512 tk]; vAB[c]: [128 tk, jt(4)*head(8)*128]
        kT = [[None] * NCH for _ in range(NPAIR)]
        for p in range(NPAIR):
            for c in range(NCH):
                kT[p][c] = cst.tile([128, 512], BF16, tag=f"k{p}{c}", name=f"k{p}{c}")
        vAB = []
        for c in range(NCH):
            v_t = cst.tile([128, 4 * 8 * 128], BF16, tag=f"v{c}", name=f"v{c}")
            vAB.append(v_t)
            # ones columns 64:128 of each [*, jt, h, 128] block
            nc.vector.memset(v_t[:].rearrange("q (a b) -> q a b", b=128)[:, :, 64:128], 1.0)

        def proj_qk(w_s, p, c, dest):
            acc = psA.tile([128, 512], F32, tag="acc", name="acc")
            for cc in range(NCC):
                nc.tensor.matmul(
                    acc[:],
                    w_s[:, cc * HH + p * 128: cc * HH + (p + 1) * 128],
                    xts[cc][c][:],
                    start=(cc == 0),
                    stop=(cc == NCC - 1),
                )
            nc.vector.tensor_copy(dest, acc[:])

        def proj_v(c):
            for jt in range(4):
                acc = psA.tile([128, 512], F32, tag="acc", name="acc")
                for cc in range(NCC):
                    nc.tensor.matmul(
                        acc[:],
                        xts[cc][c][:, jt * 128:(jt + 1) * 128],
                        wv_s[:, cc * HH:(cc + 1) * HH],
                        start=(cc == 0),
                        stop=(cc == NCC - 1),
                    )
                dst = vAB[c][:].rearrange("q (a b c2) -> q a b c2", a=4, b=8)[:, jt, :, 0:64]
                nc.vector.tensor_copy(dst, acc[:])

        def proj_chunk_kv(c):
            for p in range(NPAIR):
                proj_qk(wk_s, p, c, kT[p][c][:])
            proj_v(c)

        def proj_chunk_q(jq):
            qs = []
            for p in range(NPAIR):
                qt = qp.tile([128, 512], BF16, tag="q", name="qt")
                proj_qk(wq_s, p, jq, qt[:])
                qs.append(qt)
            return qs

        def wo_emit(jq, att_list):
            for ch in range(2):
                for tt in range(4):
                    y_ps = psA.tile([128, 512], F32, tag="acc", name="y_ps")
                    for p in range(NPAIR):
                        nc.tensor.matmul(
                            y_ps[:],
                            att_list[p][:, tt * 128:(tt + 1) * 128],
                            wo_s[:, (p * 2 + ch) * 512:(p * 2 + ch + 1) * 512],
                            start=(p == 0),
                            stop=(p == NPAIR - 1),
                        )
                    yb = yp.tile([128, 512], F32, tag="yb", name="yb")
                    nc.vector.tensor_copy(yb[:], y_ps[:])
                    t0 = jq * 512 + tt * 128
                    nc.sync.dma_start(
                        y[t0:t0 + 128, ch * 512:(ch + 1) * 512], yb[:]
                    )

        # Emission order doubles as scheduling priority, so PE filler work
        # (projections, Wo) is spread to where the scalar engine would
        # otherwise outpace the PE: each section emits the next chunk's
        # projections, and the last (heaviest) section absorbs kv(3), Wo(2)
        # and Wo(3).
        # Section 0 projections are emitted per-pair just-in-time so the
        # scalar exp pipeline starts as early as possible.
        qs = [None] * NPAIR
        att_saved = {}
        pending = []

        def norm_flush(item):
            att_t, stash = item
            for h, (ar, dn) in enumerate(stash):
                lt = rp.tile([64, 512], F32, tag="lt", name="lt")
                rc = rp.tile([64, 512], F32, tag="rc", name="rc")
                nc.scalar.activation(lt[:], dn[:], AF.Ln)
                nc.scalar.activation(rc[:], lt[:], AF.Exp, scale=-1.0)
                nc.vector.tensor_tensor(
                    att_t[h * 64:(h + 1) * 64, :], ar[:], rc[:], ALU.mult
                )

        for jq in range(NCH):
            att_cur = []
            for p in range(NPAIR):
                if jq == 0:
                    proj_qk(wk_s, p, 0, kT[p][0][:])
                    qt = qp.tile([128, 512], BF16, tag="q", name="qt")
                    proj_qk(wq_s, p, 0, qt[:])
                    qs[p] = qt
                    if p == 0:
                        proj_v(0)
                if jq == NCH - 1 and p == 2:
                    wo_emit(NCH - 2, att_saved.pop(NCH - 2))
                if len(pending) > 1:
                    norm_flush(pending.pop(0))
                paA = psPA.tile([128, 512], F32, tag="paA", name="paA")
                paB = psPA.tile([128, 512], F32, tag="paB", name="paB")
                att_t = ap_.tile([128, 512], BF16, tag="att", name="att_t")
                # chunks: (tile, col0, ncol, e_off) - diagonal tiles trimmed
                # to their causal column range [128d, 512) plus a [128,128]
                # triangle mask on the first 128 columns.
                chunks = [
                    ([(2 * m, 0, 512, 0), (2 * m + 1, 0, 512, 512)], 1024, False)
                    for m in range(2 * jq)
                ]
                t0_ = 4 * jq
                chunks.append(
                    ([(t0_, 0, 512, 0), (t0_ + 1, 128, 384, 512)], 896, True)
                )
                chunks.append(
                    ([(t0_ + 2, 256, 256, 0), (t0_ + 3, 384, 128, 256)], 384, True)
                )
                nchk = len(chunks)
                for ci, (parts, width, diag) in enumerate(chunks):
                    if jq == NCH - 1 and p == 0 and ci == nchk - 2:
                        proj_chunk_kv(NCH - 1)  # filler for the heavy section
                    for h in range(2):
                        pa = paA if h == 0 else paB
                        s = psS.tile([128, width], F32, tag="s", name="s")
                        for tl, col0, ncol, eo in parts:
                            nc.tensor.matmul(
                                s[:, eo:eo + ncol],
                                kT[p][tl // 4][h * 64:(h + 1) * 64,
                                               (tl % 4) * 128:(tl % 4 + 1) * 128],
                                qs[p][h * 64:(h + 1) * 64, col0:col0 + ncol],
                                start=True,
                                stop=True,
                            )
                        e = ep.tile([128, width], BF16, tag="e", name="e")
                        nc.scalar.activation(e[:], s[:], AF.Exp, scale=0.125)
                        if diag:
                            # zero each tile's intra-tile upper triangle
                            # (its first 128 columns)
                            for _, _, _, eo in parts:
                                nc.vector.tensor_tensor(
                                    e[:, eo:eo + 128], e[:, eo:eo + 128],
                                    tri_s[:], ALU.mult,
                                )
                        for pi, (tl, col0, ncol, eo) in enumerate(parts):
                            vsl = vAB[tl // 4][:].rearrange(
                                "q (a b c2) -> q a b c2", a=4, b=8
                            )[:, tl % 4, p * 2 + h, :]
                            nc.tensor.matmul(
                                pa[:, col0:col0 + ncol],
                                vsl,
                                e[:, eo:eo + ncol],
                                start=(ci == 0 and pi == 0),
                                stop=(ci == nchk - 1 and tl == t0_ + 3),
                            )
                # Stash raw att + denominator to SBUF with two fast DVE
                # copies so the pa psum slots free in ~1.4us (the next pair's
                # PV is serialized on them). The actual normalize
                # (r = exp(-ln(d)) on ACT, then DVE multiply) is emitted one
                # pair later so it slots into the ACT FIFO without stalling
                # the attention pipeline.
                stash = []
                for h, pa in ((0, paA), (1, paB)):
                    ar = rp.tile([64, 512], BF16, tag="ar", name="ar", bufs=4)
                    dn = rp.tile([64, 512], F32, tag="dn", name="dn", bufs=4)
                    nc.vector.tensor_copy(ar[:], pa[0:64, :])
                    nc.vector.tensor_copy(dn[:], pa[64:128, :])
                    stash.append((ar, dn))
                pending.append((att_t, stash))
                att_cur.append(att_t)

            while pending:
                norm_flush(pending.pop(0))

            if jq + 1 < NCH:
                qs = proj_chunk_q(jq + 1)
                if jq + 1 < NCH - 1:
                    proj_chunk_kv(jq + 1)  # kv(3) deferred into section 3

            if jq == NCH - 2:
                att_saved[jq] = att_cur  # Wo(2) emitted inside section 3
            else:
                wo_emit(jq, att_cur)

    _split_matmul_waits(nc)
    return nc


def _split_matmul_waits(nc):
    """walrus's fused-LDW matmul lowering can't carry multiple sync waits
    (S3_LW setupSyncWait assert). Move every matmul's waits onto a
    preceding same-engine NoOp, which lowers with full sync support."""
    f = nc.m.functions[0]
    k = 0
    for bb in f.blocks:
        insts = bb.instructions
        out = []
        for i in insts:
            waits = list(i.sync_info.on_wait) if i.sync_info is not None else []
            keep = 0 if type(i).__name__ == "InstMatmult" else 1
            if len(waits) > keep:
                moved, kept = waits[: len(waits) - keep], waits[len(waits) - keep:]
                for w in moved:
                    n = mybir.InstNoOp(name=f"I-mmwait{k}")
                    k += 1
                    n.engine = i.engine
                    n.sync_info = mybir.SyncInfo(on_wait=[w], on_update=[])
                    nc.register_instruction(n)
                    out.append(n)
                i.sync_info = mybir.SyncInfo(
                    on_wait=kept, on_update=list(i.sync_info.on_update)
                )
            out.append(i)
        if k:
            bb.instructions = out
    return nc


def _get_program():
    global _PROGRAM
    if _PROGRAM is None:
        _PROGRAM = _build_program()
    return _PROGRAM


_RUNNER = None


def _get_runner():
    """Compile the SPMD program into a cached sharded jit callable."""
    global _RUNNER
    if _RUNNER is not None:
        return _RUNNER
    import jax
    from jax.experimental.shard_map import shard_map
    from jax.sharding import Mesh, PartitionSpec

    nc = _get_program()
    install_neuronx_cc_hook()

    partition_name = (
        nc.partition_id_tensor.name if nc.partition_id_tensor else None
    )
    in_names, out_names, out_avals = [], [], []
    for alloc in nc.m.functions[0].allocations:
        if not isinstance(alloc, mybir.MemoryLocationSet):
            continue
        name = alloc.memorylocations[0].name
        if alloc.kind == "ExternalInput":
            if name != partition_name:
                in_names.append(name)
        elif alloc.kind == "ExternalOutput":
            out_names.append(name)
            out_avals.append(
                jax.core.ShapedArray(tuple(alloc.tensor_shape), mybir.dt.np(alloc.dtype))
            )
    n_params = len(in_names)
    zero_outs = [np.zeros(a.shape, a.dtype) for a in out_avals]
    all_in_names = list(in_names) + list(out_names)
    if partition_name is not None:
        all_in_names.append(partition_name)
    all_in_names = tuple(all_in_names)

    def _body(*args):
        operands = list(args)
        if partition_name is not None:
            from concourse.bass2jax import partition_id_tensor

            operands.append(partition_id_tensor())
        outs = _bass_exec_p.bind(
            *operands,
            out_avals=tuple(out_avals),
            in_names=all_in_names,
            out_names=tuple(out_names),
            lowering_input_output_aliases=(),
            sim_require_finite=True,
            sim_require_nnan=True,
            nc=nc,
        )
        return tuple(outs)

    devices = jax.devices()[:NCORES]
    assert len(devices) == NCORES, devices
    mesh = Mesh(np.asarray(devices), ("core",))
    n_all = n_params + len(out_names)
    sharded = jax.jit(
        shard_map(
            _body,
            mesh=mesh,
            in_specs=(PartitionSpec("core"),) * n_all,
            out_specs=(PartitionSpec("core"),) * len(out_names),
            check_rep=False,
        ),
        keep_unused=True,
    )
    _RUNNER = dict(
        sharded=sharded,
        in_names=in_names,
        out_names=out_names,
        out_avals=out_avals,
        zero_outs=zero_outs,
        mesh=mesh,
    )
    return _RUNNER


def _run(in_maps):
    r = _get_runner()
    concat_in = [
        np.concatenate([np.asarray(m[name]) for m in in_maps], axis=0)
        for name in r["in_names"]
    ]
    concat_zeros = [
        np.zeros((NCORES * z.shape[0], *z.shape[1:]), z.dtype) for z in r["zero_outs"]
    ]
    out_arrs = r["sharded"](*concat_in, *concat_zeros)
    return [
        {
            name: np.asarray(out_arrs[i]).reshape(NCORES, *r["out_avals"][i].shape)[c]
            for i, name in enumerate(r["out_names"])
        }
        for c in range(NCORES)
    ]


def timed_run(in_maps, iters=10):
    """Execute with inputs pre-staged on device; return per-iteration seconds."""
    import time
    import jax

    r = _get_runner()
    concat_in = [
        np.concatenate([np.asarray(m[name]) for m in in_maps], axis=0)
        for name in r["in_names"]
    ]
    concat_zeros = [
        np.zeros((NCORES * z.shape[0], *z.shape[1:]), z.dtype) for z in r["zero_outs"]
    ]
    from jax.sharding import NamedSharding, PartitionSpec

    sh = NamedSharding(r["mesh"], PartitionSpec("core"))
    args = [jax.device_put(a, sh) for a in concat_in + concat_zeros]
    out = r["sharded"](*args)  # warmup + compile
    jax.block_until_ready(out)
    times = []
    for _ in range(iters):
        t0 = time.perf_counter()
        out = r["sharded"](*args)
        jax.block_until_ready(out)
        times.append(time.perf_counter() - t0)
    return times


def bf16(a):
    return np.asarray(a, dtype=np.float32).astype(ml_dtypes.bfloat16)


def make_masks():
    pidx = np.arange(128)[:, None]
    fidx = np.arange(128)[None, :]
    return bf16((pidx <= fidx).astype(np.float32))


def make_in_maps(x, Wq, Wk, Wv, Wo):
    x = np.asarray(x, dtype=np.float32)
    Wq = np.asarray(Wq, dtype=np.float32)
    Wk = np.asarray(Wk, dtype=np.float32)
    Wv = np.asarray(Wv, dtype=np.float32)
    Wo = np.asarray(Wo, dtype=np.float32)
    mk = make_masks()
    xTs = [bf16(np.ascontiguousarray(x[b].T)) for b in range(B)]

    def wslices(W, sl):
        # [C, 512] (c, d) -> [128, cc(8), 512] -> [128, 4096]
        wt = np.ascontiguousarray(W[sl, :].T)  # torch linear: y = x @ W.T
        return bf16(wt.reshape(NCC, 128, HH).transpose(1, 0, 2).reshape(128, NCC * HH))

    in_maps = []
    for core in range(NCORES):
        b, hh = core // 2, core % 2
        sl = slice(hh * HH, (hh + 1) * HH)
        woT = np.ascontiguousarray(Wo[:, sl].T)  # [512 hd, 1024 c]
        wo_t = bf16(
            woT.reshape(NPAIR, 128, 2, 512).transpose(1, 0, 2, 3).reshape(128, NPAIR * 1024)
        )
        in_maps.append({
            "xT": xTs[b],
            "wq": wslices(Wq, sl),
            "wk": wslices(Wk, sl),
            "wv": wslices(Wv, sl),
            "wo": wo_t,
            "mk": mk,
        })
    return in_maps


def kernel(x, Wq, Wk, Wv, Wo):
    in_maps = make_in_maps(x, Wq, Wk, Wv, Wo)
    results = _run(in_maps)
    out = np.empty((B, T, C), dtype=np.float32)
    for b in range(B):
        out[b] = results[2 * b]["y"] + results[2 * b + 1]["y"]
    return out


# revision 28
# speedup vs baseline: 1.0202x; 1.0070x over previous
"""Causal multi-head attention (B=4, T=2048, C=1024, H=16, HD=64) on 8 trn2 cores.

Sharding: core i -> batch b = i//2, head-half hh = i%2 (8 heads = 512 dims).
Each core computes q/k/v projections for its 512 head-dims, causal attention
for its 8 heads, and its partial of the output projection (Wo column block).
Host sums the two head-half partials per batch.

Dataflow (all matmul operands bf16, PSUM accumulation f32):
  - x^T resident in SBUF as 8 [128c, 2048t] chunks (bf16).
  - q/k projected per (pair, t-chunk) into [128d, 512t] tiles; v projected
    directly in [t, d] layout ([128t, 512d] psum tiles) and scattered into
    vAB[chunk] = [128tk, jt(4), head(8), 128] where cols 0:64 hold v and
    cols 64:128 hold ones (so the PV matmul also emits the softmax
    denominator broadcast across psum partitions 64:128).
  - Attention runs query-chunk (jq) outer, S-transposed: S[tk, tq] psum
    [128, 1024] tiles (2 tk-tiles x 1 head), exp on ACT (scale=1/8 fused,
    bf16 out), causal mask on the 4 diagonal tiles via DVE multiply,
    PV accumulation into pa[128, 512] (64 att dims + 64 denom rows).
    Normalize = reciprocal_approx_fast(denom) + DVE multiply -> attT tile
    (SBUF bf16, no DRAM spill).
  - Wo phase per jq (interleaved with the next chunk's projections by the
    tile scheduler): y[tt, ch] accumulated over the 4 pairs from SBUF attT.
"""

import sys
from contextlib import ExitStack

import numpy as np
import ml_dtypes

try:
    from concourse import bass, tile, mybir
except ImportError:  # pragma: no cover
    sys.path.insert(0, "/opt/trn_rl_repo")
    from concourse import bass, tile, mybir

from concourse.bass2jax import _bass_exec_p, install_neuronx_cc_hook

F32 = mybir.dt.float32
BF16 = mybir.dt.bfloat16
AF = mybir.ActivationFunctionType
ALU = mybir.AluOpType

B, T, C = 4, 2048, 1024
H, HD = 16, 64
NCORES = 8
HH = 512          # head-dims per core (8 heads)
NPAIR = 4         # head-pairs per core (128 dims each)
NCC = C // 128    # 8 contraction chunks for projections
NCH = T // 512    # 4 t/tk chunks of 512
NTT = T // 128    # 16 t-tiles

_PROGRAM = None


def _build_program():
    nc = bass.Bass("TRN2", target_bir_lowering=False, debug=False)

    xT_d = nc.declare_dram_parameter("xT", [C, T], BF16, isOutput=False)
    wq_d = nc.declare_dram_parameter("wq", [128, NCC * HH], BF16, isOutput=False)
    wk_d = nc.declare_dram_parameter("wk", [128, NCC * HH], BF16, isOutput=False)
    wv_d = nc.declare_dram_parameter("wv", [128, NCC * HH], BF16, isOutput=False)
    wo_d = nc.declare_dram_parameter("wo", [128, NPAIR * 2 * 512], BF16, isOutput=False)
    mk_d = nc.declare_dram_parameter("mk", [128, 128], BF16, isOutput=False)
    y_d = nc.declare_dram_parameter("y", [T, C], F32, isOutput=True)

    xT = xT_d.ap()
    y = y_d.ap()

    with tile.TileContext(nc) as tc, ExitStack() as ctx:
        cst = ctx.enter_context(tc.tile_pool(name="cst", bufs=1))
        qp = ctx.enter_context(tc.tile_pool(name="qp", bufs=8))
        ap_ = ctx.enter_context(tc.tile_pool(name="ap", bufs=12))
        ep = ctx.enter_context(tc.tile_pool(name="ep", bufs=6))
        rp = ctx.enter_context(tc.tile_pool(name="rp", bufs=2))
        yp = ctx.enter_context(tc.tile_pool(name="yp", bufs=2))
        psS = ctx.enter_context(tc.tile_pool(name="psS", bufs=2, space="PSUM"))
        psPA = ctx.enter_context(tc.tile_pool(name="psPA", bufs=1, space="PSUM"))
        psA = ctx.enter_context(tc.tile_pool(name="psA", bufs=2, space="PSUM"))

        # --- static SBUF: weights, masks, x chunks, k/v tiles
        wq_s = cst.tile([128, NCC * HH], BF16, tag="wq_s")
        wk_s = cst.tile([128, NCC * HH], BF16, tag="wk_s")
        wv_s = cst.tile([128, NCC * HH], BF16, tag="wv_s")
        wo_s = cst.tile([128, NPAIR * 2 * 512], BF16, tag="wo_s")
        tri_s = cst.tile([128, 128], BF16, tag="tri_s")
        for sb, dr in ((wq_s, wq_d), (wk_s, wk_d), (wv_s, wv_d), (wo_s, wo_d), (tri_s, mk_d)):
            nc.sync.dma_start(sb[:], dr.ap()[:])

        # x chunks split per (row-chunk cc, t-chunk c) so projections can
        # start as soon as the columns they touch have landed. DMA order:
        # t-chunk NCH-1 first (q proj of the heaviest query chunk, which runs
        # first), then 0, 1, 2 (k/v in tk order).
        xts = [[None] * NCH for _ in range(NCC)]

        def x_dma(c):
            for cc in range(NCC):
                xt = cst.tile([128, 512], BF16, tag=f"x{cc}_{c}", name=f"x{cc}_{c}")
                # issued from GpSimd so descriptor generation runs in
                # parallel with the Sync engine's weight DMAs
                nc.gpsimd.dma_start(
                    xt[:], xT[cc * 128:(cc + 1) * 128, c * 512:(c + 1) * 512]
                )
                xts[cc][c] = xt

        x_dma(0)

        # kT[p][c]: [128 d-pair, 512 tk]; vAB[c]: [128 tk, jt(4)*head(8)*128]
        kT = [[None] * NCH for _ in range(NPAIR)]
        for p in range(NPAIR):
            for c in range(NCH):
                kT[p][c] = cst.tile([128, 512], BF16, tag=f"k{p}{c}", name=f"k{p}{c}")
        vAB = []
        for c in range(NCH):
            v_t = cst.tile([128, 4 * 8 * 128], BF16, tag=f"v{c}", name=f"v{c}")
            vAB.append(v_t)
            # ones columns 64:128 of each [*, jt, h, 128] block; on GpSimd so
            # the DVE is free for the startup k/q psum->SBUF copies
            nc.gpsimd.memset(v_t[:].rearrange("q (a b) -> q a b", b=128)[:, :, 64:128], 1.0)
        for c in range(1, NCH):
            x_dma(c)

        def proj_qk(w_s, p, c, dest):
            acc = psA.tile([128, 512], F32, tag="acc", name="acc")
            for cc in range(NCC):
                nc.tensor.matmul(
                    acc[:],
                    w_s[:, cc * HH + p * 128: cc * HH + (p + 1) * 128],
                    xts[cc][c][:],
                    start=(cc == 0),
                    stop=(cc == NCC - 1),
                )
            nc.vector.tensor_copy(dest, acc[:])

        def proj_v(c, jts=(0, 1, 2, 3)):
            for jt in jts:
                acc = psA.tile([128, 512], F32, tag="acc", name="acc")
                for cc in range(NCC):
                    nc.tensor.matmul(
                        acc[:],
                        xts[cc][c][:, jt * 128:(jt + 1) * 128],
                        wv_s[:, cc * HH:(cc + 1) * HH],
                        start=(cc == 0),
                        stop=(cc == NCC - 1),
                    )
                dst = vAB[c][:].rearrange("q (a b c2) -> q a b c2", a=4, b=8)[:, jt, :, 0:64]
                nc.vector.tensor_copy(dst, acc[:])

        def proj_chunk_kv(c):
            for p in range(NPAIR):
                proj_qk(wk_s, p, c, kT[p][c][:])
            proj_v(c)

        def proj_chunk_q(jq):
            qs = []
            for p in range(NPAIR):
                qt = qp.tile([128, 512], BF16, tag="q", name="qt")
                proj_qk(wq_s, p, jq, qt[:])
                qs.append(qt)
            return qs

        def wo_emit(jq, att_list):
            for ch in range(2):
                for tt in range(4):
                    y_ps = psA.tile([128, 512], F32, tag="acc", name="y_ps")
                    for p in range(NPAIR):
                        nc.tensor.matmul(
                            y_ps[:],
                            att_list[p][:, tt * 128:(tt + 1) * 128],
                            wo_s[:, (p * 2 + ch) * 512:(p * 2 + ch + 1) * 512],
                            start=(p == 0),
                            stop=(p == NPAIR - 1),
                        )
                    yb = yp.tile([128, 512], F32, tag="yb", name="yb")
                    nc.vector.tensor_copy(yb[:], y_ps[:])
                    t0 = jq * 512 + tt * 128
                    nc.sync.dma_start(
                        y[t0:t0 + 128, ch * 512:(ch + 1) * 512], yb[:]
                    )

        # Emission order doubles as scheduling priority, so PE filler work
        # (projections, Wo) is spread to where the scalar engine would
        # otherwise outpace the PE: each section emits the next chunk's
        # projections, and the last (heaviest) section absorbs kv(3), Wo(2)
        # and Wo(3).
        # Section 0 projections are emitted per-pair just-in-time so the
        # scalar exp pipeline starts as early as possible.
        qs = [None] * NPAIR
        att_saved = {}
        pending = []

        def norm_flush(item):
            att_t, stash = item
            for h, (ar, dn) in enumerate(stash):
                lt = rp.tile([64, 512], F32, tag="lt", name="lt")
                rc = rp.tile([64, 512], F32, tag="rc", name="rc")
                nc.scalar.activation(lt[:], dn[:], AF.Ln)
                nc.scalar.activation(rc[:], lt[:], AF.Exp, scale=-1.0)
                nc.vector.tensor_tensor(
                    att_t[h * 64:(h + 1) * 64, :], ar[:], rc[:], ALU.mult
                )

        for jq in range(NCH):
            att_cur = []
            for p in range(NPAIR):
                if jq == 0:
                    proj_qk(wk_s, p, 0, kT[p][0][:])
                    qt = qp.tile([128, 512], BF16, tag="q", name="qt")
                    proj_qk(wq_s, p, 0, qt[:])
                    qs[p] = qt
                    if p == 0:
                        proj_v(0, (0, 1))
                if jq == NCH - 1 and p == 2:
                    wo_emit(NCH - 2, att_saved.pop(NCH - 2))
                if len(pending) > 1:
                    norm_flush(pending.pop(0))
                paA = psPA.tile([128, 512], F32, tag="paA", name="paA")
                paB = psPA.tile([128, 512], F32, tag="paB", name="paB")
                att_t = ap_.tile([128, 512], BF16, tag="att", name="att_t")
                # chunks: (tile, col0, ncol, e_off) - diagonal tiles trimmed
                # to their causal column range [128d, 512) plus a [128,128]
                # triangle mask on the first 128 columns.
                chunks = [
                    ([(2 * m, 0, 512, 0), (2 * m + 1, 0, 512, 512)], 1024, False)
                    for m in range(2 * jq)
                ]
                t0_ = 4 * jq
                chunks.append(
                    ([(t0_, 0, 512, 0), (t0_ + 1, 128, 384, 512)], 896, True)
                )
                chunks.append(
                    ([(t0_ + 2, 256, 256, 0), (t0_ + 3, 384, 128, 256)], 384, True)
                )
                nchk = len(chunks)
                for ci, (parts, width, diag) in enumerate(chunks):
                    if jq == 0 and p == 0 and ci == 1:
                        proj_v(0, (2, 3))
                    if jq == NCH - 1 and p == 0 and ci == nchk - 2:
                        proj_chunk_kv(NCH - 1)  # filler for the heavy section
                    for h in range(2):
                        pa = paA if h == 0 else paB
                        s = psS.tile([128, width], F32, tag="s", name="s")
                        for tl, col0, ncol, eo in parts:
                            nc.tensor.matmul(
                                s[:, eo:eo + ncol],
                                kT[p][tl // 4][h * 64:(h + 1) * 64,
                                               (tl % 4) * 128:(tl % 4 + 1) * 128],
                                qs[p][h * 64:(h + 1) * 64, col0:col0 + ncol],
                                start=True,
                                stop=True,
                            )
                        e = ep.tile([128, width], BF16, tag="e", name="e")
                        nc.scalar.activation(e[:], s[:], AF.Exp, scale=0.125)
                        if diag:
                            # zero each tile's intra-tile upper triangle
                            # (its first 128 columns)
                            for _, _, _, eo in parts:
                                nc.vector.tensor_tensor(
                                    e[:, eo:eo + 128], e[:, eo:eo + 128],
                                    tri_s[:], ALU.mult,
                                )
                        for pi, (tl, col0, ncol, eo) in enumerate(parts):
                            vsl = vAB[tl // 4][:].rearrange(
                                "q (a b c2) -> q a b c2", a=4, b=8
                            )[:, tl % 4, p * 2 + h, :]
                            nc.tensor.matmul(
                                pa[:, col0:col0 + ncol],
                                vsl,
                                e[:, eo:eo + ncol],
                                start=(ci == 0 and pi == 0),
                                stop=(ci == nchk - 1 and tl == t0_ + 3),
                            )
                # Stash raw att + denominator to SBUF with two fast DVE
                # copies so the pa psum slots free in ~1.4us (the next pair's
                # PV is serialized on them). The actual normalize
                # (r = exp(-ln(d)) on ACT, then DVE multiply) is emitted one
                # pair later so it slots into the ACT FIFO without stalling
                # the attention pipeline.
                stash = []
                for h, pa in ((0, paA), (1, paB)):
                    ar = rp.tile([64, 512], BF16, tag="ar", name="ar", bufs=4)
                    dn = rp.tile([64, 512], F32, tag="dn", name="dn", bufs=4)
                    nc.vector.tensor_copy(ar[:], pa[0:64, :])
                    nc.vector.tensor_copy(dn[:], pa[64:128, :])
                    stash.append((ar, dn))
                pending.append((att_t, stash))
                att_cur.append(att_t)

            while pending:
                norm_flush(pending.pop(0))

            if jq + 1 < NCH:
                qs = proj_chunk_q(jq + 1)
                if jq + 1 < NCH - 1:
                    proj_chunk_kv(jq + 1)  # kv(3) deferred into section 3

            if jq == NCH - 2:
                att_saved[jq] = att_cur  # Wo(2) emitted inside section 3
            else:
                wo_emit(jq, att_cur)

    _split_matmul_waits(nc)
    return nc


def _split_matmul_waits(nc):
    """walrus's fused-LDW matmul lowering can't carry multiple sync waits
    (S3_LW setupSyncWait assert). Move every matmul's waits onto a
    preceding same-engine NoOp, which lowers with full sync support."""
    f = nc.m.functions[0]
    k = 0
    for bb in f.blocks:
        insts = bb.instructions
        out = []
        for i in insts:
            waits = list(i.sync_info.on_wait) if i.sync_info is not None else []
            keep = 0 if type(i).__name__ == "InstMatmult" else 1
            if len(waits) > keep:
                moved, kept = waits[: len(waits) - keep], waits[len(waits) - keep:]
                for w in moved:
                    n = mybir.InstNoOp(name=f"I-mmwait{k}")
                    k += 1
                    n.engine = i.engine
                    n.sync_info = mybir.SyncInfo(on_wait=[w], on_update=[])
                    nc.register_instruction(n)
                    out.append(n)
                i.sync_info = mybir.SyncInfo(
                    on_wait=kept, on_update=list(i.sync_info.on_update)
                )
            out.append(i)
        if k:
            bb.instructions = out
    return nc


def _get_program():
    global _PROGRAM
    if _PROGRAM is None:
        _PROGRAM = _build_program()
    return _PROGRAM


_RUNNER = None


def _get_runner():
    """Compile the SPMD program into a cached sharded jit callable."""
    global _RUNNER
    if _RUNNER is not None:
        return _RUNNER
    import jax
    from jax.experimental.shard_map import shard_map
    from jax.sharding import Mesh, PartitionSpec

    nc = _get_program()
    install_neuronx_cc_hook()

    partition_name = (
        nc.partition_id_tensor.name if nc.partition_id_tensor else None
    )
    in_names, out_names, out_avals = [], [], []
    for alloc in nc.m.functions[0].allocations:
        if not isinstance(alloc, mybir.MemoryLocationSet):
            continue
        name = alloc.memorylocations[0].name
        if alloc.kind == "ExternalInput":
            if name != partition_name:
                in_names.append(name)
        elif alloc.kind == "ExternalOutput":
            out_names.append(name)
            out_avals.append(
                jax.core.ShapedArray(tuple(alloc.tensor_shape), mybir.dt.np(alloc.dtype))
            )
    n_params = len(in_names)
    zero_outs = [np.zeros(a.shape, a.dtype) for a in out_avals]
    all_in_names = list(in_names) + list(out_names)
    if partition_name is not None:
        all_in_names.append(partition_name)
    all_in_names = tuple(all_in_names)

    def _body(*args):
        operands = list(args)
        if partition_name is not None:
            from concourse.bass2jax import partition_id_tensor

            operands.append(partition_id_tensor())
        outs = _bass_exec_p.bind(
            *operands,
            out_avals=tuple(out_avals),
            in_names=all_in_names,
            out_names=tuple(out_names),
            lowering_input_output_aliases=(),
            sim_require_finite=True,
            sim_require_nnan=True,
            nc=nc,
        )
        return tuple(outs)

    devices = jax.devices()[:NCORES]
    assert len(devices) == NCORES, devices
    mesh = Mesh(np.asarray(devices), ("core",))
    n_all = n_params + len(out_names)
    sharded = jax.jit(
        shard_map(
            _body,
            mesh=mesh,
            in_specs=(PartitionSpec("core"),) * n_all,
            out_specs=(PartitionSpec("core"),) * len(out_names),
            check_rep=False,
        ),
        keep_unused=True,
    )
    _RUNNER = dict(
        sharded=sharded,
        in_names=in_names,
        out_names=out_names,
        out_avals=out_avals,
        zero_outs=zero_outs,
        mesh=mesh,
    )
    return _RUNNER


def _run(in_maps):
    r = _get_runner()
    concat_in = [
        np.concatenate([np.asarray(m[name]) for m in in_maps], axis=0)
        for name in r["in_names"]
    ]
    concat_zeros = [
        np.zeros((NCORES * z.shape[0], *z.shape[1:]), z.dtype) for z in r["zero_outs"]
    ]
    out_arrs = r["sharded"](*concat_in, *concat_zeros)
    return [
        {
            name: np.asarray(out_arrs[i]).reshape(NCORES, *r["out_avals"][i].shape)[c]
            for i, name in enumerate(r["out_names"])
        }
        for c in range(NCORES)
    ]


def timed_run(in_maps, iters=10):
    """Execute with inputs pre-staged on device; return per-iteration seconds."""
    import time
    import jax

    r = _get_runner()
    concat_in = [
        np.concatenate([np.asarray(m[name]) for m in in_maps], axis=0)
        for name in r["in_names"]
    ]
    concat_zeros = [
        np.zeros((NCORES * z.shape[0], *z.shape[1:]), z.dtype) for z in r["zero_outs"]
    ]
    from jax.sharding import NamedSharding, PartitionSpec

    sh = NamedSharding(r["mesh"], PartitionSpec("core"))
    args = [jax.device_put(a, sh) for a in concat_in + concat_zeros]
    out = r["sharded"](*args)  # warmup + compile
    jax.block_until_ready(out)
    times = []
    for _ in range(iters):
        t0 = time.perf_counter()
        out = r["sharded"](*args)
        jax.block_until_ready(out)
        times.append(time.perf_counter() - t0)
    return times


def bf16(a):
    return np.asarray(a, dtype=np.float32).astype(ml_dtypes.bfloat16)


def make_masks():
    pidx = np.arange(128)[:, None]
    fidx = np.arange(128)[None, :]
    return bf16((pidx <= fidx).astype(np.float32))


def make_in_maps(x, Wq, Wk, Wv, Wo):
    x = np.asarray(x, dtype=np.float32)
    Wq = np.asarray(Wq, dtype=np.float32)
    Wk = np.asarray(Wk, dtype=np.float32)
    Wv = np.asarray(Wv, dtype=np.float32)
    Wo = np.asarray(Wo, dtype=np.float32)
    mk = make_masks()
    xTs = [bf16(np.ascontiguousarray(x[b].T)) for b in range(B)]

    def wslices(W, sl):
        # [C, 512] (c, d) -> [128, cc(8), 512] -> [128, 4096]
        wt = np.ascontiguousarray(W[sl, :].T)  # torch linear: y = x @ W.T
        return bf16(wt.reshape(NCC, 128, HH).transpose(1, 0, 2).reshape(128, NCC * HH))

    in_maps = []
    for core in range(NCORES):
        b, hh = core // 2, core % 2
        sl = slice(hh * HH, (hh + 1) * HH)
        woT = np.ascontiguousarray(Wo[:, sl].T)  # [512 hd, 1024 c]
        wo_t = bf16(
            woT.reshape(NPAIR, 128, 2, 512).transpose(1, 0, 2, 3).reshape(128, NPAIR * 1024)
        )
        in_maps.append({
            "xT": xTs[b],
            "wq": wslices(Wq, sl),
            "wk": wslices(Wk, sl),
            "wv": wslices(Wv, sl),
            "wo": wo_t,
            "mk": mk,
        })
    return in_maps


def kernel(x, Wq, Wk, Wv, Wo):
    in_maps = make_in_maps(x, Wq, Wk, Wv, Wo)
    results = _run(in_maps)
    out = np.empty((B, T, C), dtype=np.float32)
    for b in range(B):
        out[b] = results[2 * b]["y"] + results[2 * b + 1]["y"]
    return out
